# revision 1
# baseline (speedup 1.0000x reference)
"""Trainium2 Bass kernel for SAGAN-style self-attention (nn_Attention_13056700580138).

Reference computation (per batch element, with N = H*W = 4096, C = 256, CK = 32):
    f  = x @ Wf + bf            [N, CK]
    g  = x @ Wg + bg            [N, CK]
    hh = x @ Wh + bh            [N, C]
    S  = g @ f^T                [N, N]
    A  = softmax(S, axis=-1)
    o  = A @ hh                 [N, C]
    out = gamma * (o @ Wo + bo) + x

Sharding: data-parallel over batch - one batch element per NeuronCore (B = 8 = n_cores).

Per-core strategy:
  * All matmuls run as float32r (FP22 operand reads, fp32 accumulate): 1 cycle/row
    on the PE at moving-dim >= 256 - 4x faster than true fp32.
  * Output projection is folded through associativity:
        (A @ hh) @ Wo + bo = A @ (x @ (Wh @ Wo) + (bh @ Wo + bo))
    (softmax rows sum to 1, so the row-bias passes through A exactly). Wh @ Wo and
    bh @ Wo + bo are computed once on-chip; the epilogue collapses to one
    reciprocal + one fused multiply-add per output block. No transposes of o.
  * Scores are computed transposed (S^T tiles [128 keys, 512 queries]) so the
    exp'd tiles feed the A @ hw accumulation directly as stationary operands.
  * The CK=32 score contraction would idle 3/4 of the PE array, so 4 key blocks
    run concurrently in separate tile_position row groups (full array).
  * Softmax needs no max-subtraction (|scores| < ~60 by construction, exp fits
    fp32) and no N x N normalize pass: an all-ones column appended to the value
    matrix makes the same accumulation emit the softmax row-sums; one reciprocal
    + fused multiply-add per [128, 256] output block finishes softmax+residual.
  * x^T (the stationary side of the f/g/hw projections) is produced by 64 PE
    transpose-mode matmuls against an identity, pipelined with DVE/ACT copies
    and the f/g projections per 512-pixel slice.
  * hw1 (the value matrix) is emitted in attention consumption order (kb = 8t+g4)
    so the attention loop starts before the prologue fully drains.
"""

from contextlib import ExitStack

import numpy as np

import bass_rust
import concourse.bass as bass
import concourse.mybir as mybir
import concourse.tile as tile
from concourse.bass_utils import run_bass_kernel_spmd
from concourse.masks import make_identity
from concourse.vector_clock import ScopedClock

FP = mybir.dt.float32
FPR = mybir.dt.float32r
AF = mybir.ActivationFunctionType
ALU = mybir.AluOpType

B, H, W, C = 8, 64, 64, 256
CK = C // 8
N = H * W  # 4096
NCORES = 8


# --- workaround: walrus in this toolchain lowers at most one sync-wait per SP
# CTRL instruction, but TileContext's final drain carries one wait per busy
# processor. Split them across single-wait carrier nops (same engine queue,
# program order => identical semantics).
def _split_drain_and_barrier(self, tick_clock, wait_clock):
    nc = self.nc
    ticks = list(eval(repr(tick_clock.global_clock).replace("VectorClock", "")))
    nproc = len(ticks)
    for i, t in enumerate(ticks):
        if t > 0:
            sub = [0] * nproc
            sub[i] = t
            carrier = nc.sync.nop(nofuse=True, hint="drain_split_wait")
            wait_clock.add_sem_waits(
                carrier.ins, ScopedClock({None: bass_rust.VectorClock(sub)})
            )
    nc.sync.drain()
    nc.all_engine_barrier()
    assert self.sems is not None
    popped = nc._tile_sem_poison_stack.pop()
    assert popped is self._sem_poison
    nc.clear_and_free_semaphores(list(self.sems.allocated().values()))
    nc.all_engine_barrier()


tile.TileContext._drain_and_barrier = _split_drain_and_barrier


def _split_instruction_waits(nc):
    """walrus in this toolchain lowers at most one sync-wait per instruction
    for several instruction templates. After Tile scheduling, move any extra
    waits onto single-wait carrier nops inserted just before the instruction
    on the same engine queue (identical blocking semantics)."""
    cnt = 0
    for fn in nc.m.functions:
        for bb in fn.blocks:
            out = []
            changed = False
            for ins in bb.instructions:
                si = ins.sync_info
                waits = list(si.on_wait) if (si is not None and si.on_wait) else []
                if len(waits) > 1:
                    changed = True
                    for wx in waits[:-1]:
                        nop = mybir.InstNoOp(name=f"wsplit-{cnt}", ins=[], outs=[])
                        cnt += 1
                        nop.engine = ins.engine
                        nop.sync_info = mybir.SyncInfo(on_wait=[wx], on_update=[])
                        nc.register_instruction(nop, overwrite=True)
                        out.append(nop)
                    si.on_wait = [waits[-1]]
                out.append(ins)
            if changed:
                bb.instructions = out


def _emit(ctx, nc, tc, t_in, t_out):
    x_d = t_in["x"]

    singles = ctx.enter_context(tc.tile_pool(name="singles", bufs=1))
    etp = ctx.enter_context(tc.tile_pool(name="etp", bufs=6))
    work = ctx.enter_context(tc.tile_pool(name="work", bufs=4))
    pre_ctx = ExitStack()
    psum_pre = pre_ctx.enter_context(tc.tile_pool(name="psum_pre", bufs=4, space="PSUM"))

    # Wh/Wo first: the Whw precompute sits at the head of the PE queue and must
    # not head-block the x transposes behind a late weight DMA.
    wh_sb = singles.tile([128, 2, C], FP)
    wo_sb = singles.tile([128, 2, C], FPR)
    for kc in range(2):
        nc.sync.dma_start(out=wh_sb[:, kc, :], in_=t_in["Wh"][kc * 128:(kc + 1) * 128, :])
        nc.sync.dma_start(out=wo_sb[:, kc, :], in_=t_in["Wo"][kc * 128:(kc + 1) * 128, :].bitcast(FPR))

    # x, split into 8 chunks so the transposes can start on chunk 0
    x_view = x_d.ap().rearrange("(t p) c -> p t c", p=128)
    x_pix = []
    for q in range(8):
        xp = singles.tile([128, 4, C], FP, name=f"x_pix{q}")
        if q == 0:  # split the first chunk so the first transpose starts sooner
            nc.sync.dma_start(out=xp[:, 0:2, :], in_=x_view[:, 0:2, :])
            nc.sync.dma_start(out=xp[:, 2:4, :], in_=x_view[:, 2:4, :])
        else:
            nc.sync.dma_start(out=xp[:], in_=x_view[:, 4 * q:4 * (q + 1), :])
        x_pix.append(xp)

    identity = singles.tile([128, 128], FP)
    make_identity(nc, identity[:])

    # remaining weights / biases / gamma
    wf_sb = singles.tile([128, 2, CK], FPR)
    wg_sb = singles.tile([128, 2, CK], FPR)
    for kc in range(2):
        nc.sync.dma_start(out=wf_sb[:, kc, :], in_=t_in["Wf"][kc * 128:(kc + 1) * 128, :].bitcast(FPR))
        nc.sync.dma_start(out=wg_sb[:, kc, :], in_=t_in["Wg"][kc * 128:(kc + 1) * 128, :].bitcast(FPR))

    bf_rep = singles.tile([128, 1], FP)
    bg_rep = singles.tile([128, 1], FP)
    for t in range(4):
        nc.sync.dma_start(out=bf_rep[32 * t:32 * t + 32, :], in_=t_in["bf"][:].unsqueeze(1))
        nc.sync.dma_start(out=bg_rep[32 * t:32 * t + 32, :], in_=t_in["bg"][:].unsqueeze(1))

    bh_col = singles.tile([128, 2, 1], FPR)
    bo_row = singles.tile([1, C], FPR)
    for kc in range(2):
        nc.sync.dma_start(
            out=bh_col[:, kc, :], in_=t_in["bh"][kc * 128:(kc + 1) * 128].unsqueeze(1).bitcast(FPR)
        )
    nc.sync.dma_start(out=bo_row[0:1, :], in_=t_in["bo"][:].unsqueeze(0).bitcast(FPR))

    ones_f = singles.tile([1, 128], FP)
    nc.vector.memset(ones_f[:], 1.0)
    ones_col = singles.tile([1, 128], FPR)
    nc.vector.tensor_copy(out=ones_col[:], in_=ones_f[:])

    gamma_rep = singles.tile([128, 1], FP)
    nc.sync.dma_start(out=gamma_rep[0:1, :], in_=t_in["gamma"][:].unsqueeze(0))
    for step in range(7):  # doubling broadcast 1 -> 128 partitions
        w_ = 1 << step
        nc.sync.dma_start(out=gamma_rep[w_:2 * w_, :], in_=gamma_rep[0:w_, :])

    # ---------------- fused output projection: Whw = Wh @ Wo, bhw = bh@Wo + bo
    whT = singles.tile([128, 2, C], FPR)  # [m % 128, m // 128, i] = Wh[i, m]
    for mc in range(2):
        for ib in range(2):
            pt = psum_pre.tile([128, 128], FP, tag="transp", bufs=3, name=f"ptw_{mc}_{ib}")
            nc.tensor.transpose(
                pt[:], wh_sb[:, ib, 128 * mc:128 * (mc + 1)], identity[:]
            )
            nc.vector.tensor_copy(out=whT[:, mc, 128 * ib:128 * (ib + 1)], in_=pt[:])

    whw_sb = singles.tile([128, 2, C], FPR)  # [i % 128, i // 128, o]
    for ib in range(2):
        ps = psum_pre.tile([128, C], FP, tag="pre", bufs=2, name=f"psw{ib}")
        for mc in range(2):
            nc.tensor.matmul(
                ps[:],
                whT[:, mc, 128 * ib:128 * (ib + 1)],
                wo_sb[:, mc, :],
                start=(mc == 0),
                stop=(mc == 1),
            )
        nc.vector.tensor_copy(out=whw_sb[:, ib, :], in_=ps[:])

    bhw_bc = singles.tile([128, C], FP)  # (bh @ Wo + bo) broadcast to all parts
    ps_b = psum_pre.tile([1, C], FP, tag="pre", bufs=2)
    for kc in range(2):
        nc.tensor.matmul(
            ps_b[:], bh_col[:, kc, :], wo_sb[:, kc, :], start=(kc == 0), stop=False
        )
    nc.tensor.matmul(
        ps_b[:], ones_col[0:1, 0:1], bo_row[:], start=False, stop=True
    )
    nc.vector.tensor_copy(out=bhw_bc[0:1, :], in_=ps_b[:])
    for step in range(7):
        w_ = 1 << step
        nc.sync.dma_start(out=bhw_bc[w_:2 * w_, :], in_=bhw_bc[0:w_, :])

    # ---- x^T transposes pipelined with f/g projections, per 512-pixel slice
    # row group t owns keys [1024*t, 1024*(t+1)); fT4[32t:32t+32, j*128:(j+1)*128]
    # holds f^T for global key block 8t+j. gT4 replicates g^T into all 4 groups.
    xT = [[singles.tile([128, 512], FPR, name=f"xT_{kc}_{s}") for s in range(8)] for kc in range(2)]
    fT4 = singles.tile([128, 1024], FPR)
    gT4 = singles.tile([128, N], FPR)
    fT_flat = singles.tile([32, N], FPR)

    for s in range(8):
        for kb in range(4 * s, 4 * s + 4):
            for kc in range(2):
                pt = psum_pre.tile([128, 128], FP, tag="transp", name=f"pt_{kb}_{kc}", bufs=3)
                nc.tensor.transpose(
                    pt[:], x_pix[kb // 4][:, kb % 4, 128 * kc:128 * (kc + 1)], identity[:]
                )
                dst = xT[kc][s][:, 128 * (kb % 4):128 * (kb % 4 + 1)]
                if kc == 0:
                    nc.vector.tensor_copy(out=dst, in_=pt[:])
                else:
                    nc.scalar.activation(out=dst, in_=pt[:], func=AF.Identity, bias=0.0)
        psf = psum_pre.tile([32, 512], FP, tag="pre", name=f"psf{s}", bufs=2)
        for kc in range(2):
            nc.tensor.matmul(
                psf[:],
                wf_sb[:, kc, :],
                xT[kc][s][:],
                start=(kc == 0),
                stop=(kc == 1),
            )
        nc.scalar.activation(
            out=fT_flat[0:32, 512 * s:512 * (s + 1)],
            in_=psf[:],
            func=AF.Identity,
            bias=bf_rep[0:32, :],
        )
        psg = psum_pre.tile([32, 512], FP, tag="pre", name=f"psg{s}", bufs=2)
        for kc in range(2):
            nc.tensor.matmul(
                psg[:],
                wg_sb[:, kc, :],
                xT[kc][s][:],
                start=(kc == 0),
                stop=(kc == 1),
            )
        nc.scalar.activation(
            out=gT4[0:32, 512 * s:512 * (s + 1)],
            in_=psg[:],
            func=AF.Identity,
            bias=bg_rep[0:32, :],
        )

    for t in range(4):
        nc.sync.dma_start(out=fT4[32 * t:32 * t + 32, :], in_=fT_flat[0:32, 1024 * t:1024 * (t + 1)])
    for t in range(1, 4):
        nc.sync.dma_start(out=gT4[32 * t:32 * t + 32, :], in_=gT4[0:32, :])

    pre_ctx.close()
    psum_sc = ctx.enter_context(tc.tile_pool(name="psum_sc", bufs=4, space="PSUM"))
    psum_o = ctx.enter_context(tc.tile_pool(name="psum_o", bufs=4, space="PSUM"))

    # ---------------- hw1 = [x @ Whw + bhw | 1] pixel-major ------------------
    # emitted in the order attention consumes it (iteration g4 uses kb = 8t+g4),
    # and allocated from the attention score slots so early score matmuls
    # pipeline in behind hw1's progressive frees instead of the whole phase.
    hw1 = singles.tile([128, 32, C + 2], FPR)
    ones_stage = singles.tile([128, 64], FP)
    nc.vector.memset(ones_stage[:], 1.0)
    nc.vector.tensor_copy(out=hw1[:, :, C:C + 2], in_=ones_stage[:])

    def emit_hw1(g4):
        for t in range(4):
            kb = 8 * t + g4
            ps = psum_sc.tile([128, C], FP, tag="score", name=f"psh{kb}")
            for kc in range(2):
                nc.tensor.matmul(
                    ps[:],
                    xT[kc][kb // 4][:, 128 * (kb % 4):128 * (kb % 4 + 1)],
                    whw_sb[:, kc, :],
                    start=(kc == 0),
                    stop=(kc == 1),
                )
            if t % 2 == 0:
                nc.vector.tensor_add(out=hw1[:, kb, 0:C], in0=ps[:], in1=bhw_bc[:])
            else:
                nc.scalar.activation(
                    out=hw1[:, kb, 0:C], in_=ps[:], func=AF.Identity,
                    bias=0.0, scale=1.0,
                )
                nc.vector.tensor_add(out=hw1[:, kb, 0:C], in0=hw1[:, kb, 0:C], in1=bhw_bc[:])

    # ---------------- attention --------------------------------------------
    for qs in range(8):  # 512-query slices
        o_ps = [psum_o.tile([128, C + 2], FP, tag="oacc", name=f"oacc_{qs}_{j}") for j in range(4)]
        for g4 in range(8):  # one key block per row group per iteration
            et = []
            for t in range(4):
                sc = psum_sc.tile([128, 512], FP, tag="score", name=f"sc_{qs}_{g4}_{t}")
                nc.tensor.matmul(
                    sc[:],
                    fT4[32 * t:32 * t + 32, 128 * g4:128 * (g4 + 1)],
                    gT4[32 * t:32 * t + 32, 512 * qs:512 * (qs + 1)],
                    start=True,
                    stop=True,
                    tile_position=(32 * t, 0),
                )
                e = etp.tile([128, 512], FPR, tag="et", name=f"et_{qs}_{g4}_{t}")
                nc.scalar.activation(out=e[:], in_=sc[:], func=AF.Exp)
                et.append(e)
            if qs == 0:
                emit_hw1(g4)
            for t in range(4):
                kb = 8 * t + g4
                for j in range(4):
                    nc.tensor.matmul(
                        o_ps[j][:],
                        et[t][:, 128 * j:128 * (j + 1)],
                        hw1[:, kb, :],
                        start=(g4 == 0 and t == 0),
                        stop=(g4 == 7 and t == 3),
                    )
        for j in range(4):
            blk = 4 * qs + j
            rinv = work.tile([128, 1], FP, tag="rinv", name=f"rinv_{blk}")
            nc.vector.reciprocal(out=rinv[:], in_=o_ps[j][:, C:C + 1])
            nc.vector.tensor_mul(out=rinv[:], in0=rinv[:], in1=gamma_rep[:])
            out_sb = work.tile([128, C], FP, tag="outsb", name=f"osb_{blk}")
            nc.vector.scalar_tensor_tensor(
                out=out_sb[:],
                in0=o_ps[j][:, 0:C],
                scalar=rinv[:],
                in1=x_pix[blk // 4][:, blk % 4, :],
                op0=ALU.mult,
                op1=ALU.add,
            )
            nc.sync.dma_start(out=t_out[128 * blk:128 * (blk + 1), :], in_=out_sb[:])


_CACHE = {}


def _build():
    if "nc" not in _CACHE:
        nc = bass.Bass("TRN2", target_bir_lowering=False, debug=False)
        t_in = {
            "x": nc.dram_tensor("x", [N, C], FP, kind="ExternalInput"),
            "Wf": nc.dram_tensor("Wf", [C, CK], FP, kind="ExternalInput"),
            "bf": nc.dram_tensor("bf", [CK], FP, kind="ExternalInput"),
            "Wg": nc.dram_tensor("Wg", [C, CK], FP, kind="ExternalInput"),
            "bg": nc.dram_tensor("bg", [CK], FP, kind="ExternalInput"),
            "Wh": nc.dram_tensor("Wh", [C, C], FP, kind="ExternalInput"),
            "bh": nc.dram_tensor("bh", [C], FP, kind="ExternalInput"),
            "Wo": nc.dram_tensor("Wo", [C, C], FP, kind="ExternalInput"),
            "bo": nc.dram_tensor("bo", [C], FP, kind="ExternalInput"),
            "gamma": nc.dram_tensor("gamma", [1], FP, kind="ExternalInput"),
        }
        t_out = nc.dram_tensor("out", [N, C], FP, kind="ExternalOutput")
        with tile.TileContext(nc) as tc:
            with ExitStack() as ctx:
                _emit(ctx, nc, tc, t_in, t_out)
        _split_instruction_waits(nc)
        _CACHE["nc"] = nc
    return _CACHE["nc"]


def kernel(x, Wf, bf, Wg, bg, Wh, bh, Wo, bo, gamma, _trace=False, _tmpdir=None):
    nc = _build()
    x = np.ascontiguousarray(np.asarray(x, dtype=np.float32)).reshape(B, N, C)
    w = {
        "Wf": np.ascontiguousarray(np.asarray(Wf, np.float32)),
        "bf": np.ascontiguousarray(np.asarray(bf, np.float32)),
        "Wg": np.ascontiguousarray(np.asarray(Wg, np.float32)),
        "bg": np.ascontiguousarray(np.asarray(bg, np.float32)),
        "Wh": np.ascontiguousarray(np.asarray(Wh, np.float32)),
        "bh": np.ascontiguousarray(np.asarray(bh, np.float32)),
        "Wo": np.ascontiguousarray(np.asarray(Wo, np.float32)),
        "bo": np.ascontiguousarray(np.asarray(bo, np.float32)),
        "gamma": np.ascontiguousarray(np.asarray(gamma, np.float32)),
    }
    in_maps = [dict(w, x=x[i]) for i in range(NCORES)]
    res = run_bass_kernel_spmd(
        nc, in_maps, core_ids=list(range(NCORES)), trace=_trace, tmpdir=_tmpdir
    )
    out = np.stack([res.results[i]["out"] for i in range(NCORES)])
    if _trace:
        kernel._last_result = res
    return out.reshape(B, H, W, C).astype(np.float32)

